# revision 16
# baseline (speedup 1.0000x reference)
"""Trainium2 Bass kernel for nn_CrossSelfDecoder (B=4,N=1024,D=1024,H=16,F=4096).

Sharding: 8 cores = (batch b in 0..3) x (head-half hh in 0..1). Each core
computes attention for its 8 heads over all 1024 positions of its batch.
Because the reference reshapes (B,H,N,Dp)->(B,N,D) without permuting heads
back, head-ownership makes row-ownership invariant: core (b,hh) owns rows
[512*hh, 512*hh+512) of batch b through the whole network.

Design (v2):
- Host pre-transposes x1/x2 and pre-tiles all weights into bf16 DRAM
  layouts: zero device-side transposes, contiguous per-partition DMAs.
- All matmuls bf16 x bf16 with fp32 PSUM accumulate.
- Attention processed per head-PAIR (t4 = hloc//2): scores of the even/odd
  head are row-packed (K=64 tiles at array rows 0-63 / 64-127, concurrent);
  AV of the pair is column-packed into one [128,512] PSUM bank (M=64 tiles
  at array cols 0-63 / 64-127); softmax denominators are 4x column-packed
  M=1 matmuls (ones lhsT) deferred to the end of the pair's group.
- Projections are emitted interleaved with score groups so the Scalar
  engine's EXP stream (the attention pacer) starts ~40us early.
- LayerNorm statistics accumulate per 128-row band as attention groups
  complete (into spare rows of the drained AV PSUM bank); only the small
  scalar chain + apply remain as a tail.
- One pairwise AllGather of the LN1 rows (bf16) overlapped with the
  own-row halves of the k2/v2 projections; partner rows recovered as
  (block0 - own) + block1.
"""

import numpy as np
import ml_dtypes

import concourse.mybir as mybir
import concourse.tile as tile
from concourse import bacc
from concourse.bass_utils import run_bass_kernel_spmd

FP32 = mybir.dt.float32
BF16 = mybir.dt.bfloat16
AF = mybir.ActivationFunctionType
ALU = mybir.AluOpType

B, N, D, H, F = 4, 1024, 1024, 16, 4096
Dp = D // H           # 64
HPC = 8               # heads per core
PC = 128              # partition chunk
NF = 512              # free chunk (one psum bank of fp32)
KC = D // PC          # 8 contraction chunks
FT = F // PC          # 32 f-tiles
EPS = 1e-5
BF = ml_dtypes.bfloat16

_CACHE = {}


def _build():
    nc = bacc.Bacc("TRN2", target_bir_lowering=False, debug=False,
                   num_devices=8)
    dram = {}
    specs = [
        ("x2t", [D, N], BF16), ("x1t", [D, N], BF16),
        ("x2own", [PC, KC * NF], BF16),
        ("wq", [4, PC, KC, PC], BF16), ("wk", [4, PC, KC, PC], BF16),
        ("wv", [KC, PC, NF], BF16),
        ("wq2", [4, PC, KC, PC], BF16), ("wk2", [4, PC, KC, PC], BF16),
        ("wv2", [KC, PC, NF], BF16),
        ("w1", [FT, PC, KC, PC], BF16), ("w2", [KC, PC, FT, PC], BF16),
        ("bqc", [PC, 4], FP32), ("bkc", [PC, 4], FP32),
        ("bq2c", [PC, 4], FP32), ("bk2c", [PC, 4], FP32),
        ("bvr", [1, NF], FP32), ("bv2r", [1, NF], FP32),
        ("b1c", [PC, FT], FP32), ("b2c", [PC, KC], FP32),
        ("gammac", [PC, KC], FP32), ("betac", [PC, KC], FP32),
    ]
    for nm, shp, dt in specs:
        dram[nm] = nc.dram_tensor(nm, shp, dt, kind="ExternalInput")
    y_out = nc.dram_tensor("y", [D, NF], FP32, kind="ExternalOutput")
    dbg = {}
    if _CACHE.get("debug"):
        for nm in ("dxT", "dnTo", "dx3T", "dn3T"):
            dbg[nm] = nc.dram_tensor(nm, [PC, KC * NF], BF16,
                                     kind="ExternalOutput")
        for qh in range(2):
            dbg[f"dO{qh}"] = nc.dram_tensor(f"dO{qh}", [PC, NF], FP32,
                                            kind="ExternalOutput")
            dbg[f"dden{qh}"] = nc.dram_tensor(f"dden{qh}", [33, NF], FP32,
                                              kind="ExternalOutput")
            dbg[f"dpt{qh}"] = nc.dram_tensor(f"dpt{qh}", [PC, 2 * NF], BF16,
                                             kind="ExternalOutput")

    # collective staging: own LN1 rows out, both group blocks back
    ag_in = nc.dram_tensor("agin", [D, NF], BF16, kind="Internal")
    ag_out = nc.dram_tensor("agout", [2, D, NF], BF16, kind="Internal")

    with tile.TileContext(nc) as tc:
        _emit(nc, tc, dram, ag_in, ag_out, y_out, dbg)
    nc.compile()
    return nc


def _proj_T(nc, sub, psp, w_dram, bias_cols, rhs_of, out_tiles, tag,
            nf_range=(0, 2), m_range=(0, 4), wtiles=None):
    """out[m][:, nf*512:...] = (W.T @ rhs + bias), T-domain."""
    for m in range(*m_range):
        if wtiles is not None:
            wt = wtiles[m]
        else:
            wt = sub.tile([PC, KC, PC], BF16, tag=f"w_{tag}", bufs=2,
                          name=f"w_{tag}{m}")
            nc.sync.dma_start(wt[:], w_dram.ap()[m])
        for nf in range(*nf_range):
            ps = psp.tile([PC, NF], FP32, tag="proj", bufs=2,
                          name=f"proj_{tag}{m}_{nf}")
            for kc in range(KC):
                nc.tensor.matmul(ps[:], wt[:, kc, :], rhs_of(kc, nf),
                                 start=(kc == 0), stop=(kc == KC - 1))
            nc.scalar.activation(out_tiles[m][:, nf * NF:(nf + 1) * NF],
                                 ps[:], AF.Identity,
                                 bias=bias_cols[:, m:m + 1])


def _proj_v(nc, sub, psp, wts, bvB, rhs_of, v_tiles, tag, pc_range=(0, 8)):
    """v natural: v_tiles[pc] [128 keys, 512 own-head cols]."""
    for pc in range(*pc_range):
        ps = psp.tile([PC, NF], FP32, tag="proj", bufs=2, name=f"v_{tag}{pc}")
        for kc in range(KC):
            nc.tensor.matmul(ps[:], rhs_of(kc, pc), wts[kc][:],
                             start=(kc == 0), stop=(kc == KC - 1))
        nc.vector.tensor_tensor(v_tiles[pc][:], ps[:], bvB[:], op=ALU.add)


def _attn_scores(nc, sub, psp, qT, kT, t4, qh, tag):
    """Row-packed scores + exp for head pair (2t4, 2t4+1), query half qh.
    Returns the 8 pt tiles [128, 2*NF] (free: h2-half x 512 queries)."""
    pts = []
    for kc in range(KC):
        sps = psp.tile([PC, 2 * NF], FP32, tag="S", bufs=2,
                       name=f"S_{tag}{t4}_{qh}_{kc}")
        for h2 in range(2):
            r64 = Dp * h2
            nc.tensor.matmul(
                sps[:, h2 * NF:(h2 + 1) * NF],
                kT[t4][r64:r64 + Dp, kc * PC:(kc + 1) * PC],
                qT[t4][r64:r64 + Dp, qh * NF:(qh + 1) * NF],
                start=True, stop=True)
        pt = sub.tile([PC, 2 * NF], BF16, tag="PT", bufs=9,
                      name=f"PT_{tag}{t4}_{qh}_{kc}")
        nc.scalar.activation(pt[:], sps[:], AF.Exp)
        pts.append(pt)
    return pts


def _attn_group(nc, sub, psp, v_tiles, all_pts, xT, t4, c, tag,
                after_band=None, dbg=None):
    """AV (col-packed pairs) + dens (4x col-packed) + normalize + scatter
    for head pair t4 (both query halves). all_pts[qh] = pt tiles."""
    o_tiles = []
    rsbs = []
    for qh in range(2):
        ops = psp.tile([PC, NF], FP32, tag="O", bufs=2,
                       name=f"O_{tag}{t4}_{qh}")
        for kc in range(KC):
            for h2 in range(2):
                nc.tensor.matmul(
                    ops[Dp * h2:Dp * (h2 + 1), :],
                    v_tiles[kc][:, Dp * (2 * t4 + h2):
                                Dp * (2 * t4 + h2 + 1)],
                    all_pts[qh][kc][:, h2 * NF:(h2 + 1) * NF],
                    start=(kc == 0), stop=(kc == KC - 1))
        o_tiles.append(ops)
        dens = psp.tile([33, NF], FP32, tag="proj", bufs=2,
                        name=f"den_{tag}{t4}_{qh}")
        for kc in range(KC):
            for h2 in range(2):
                nc.tensor.matmul(
                    dens[32 * h2:32 * h2 + 1, :], c["onesk"][:],
                    all_pts[qh][kc][:, h2 * NF:(h2 + 1) * NF],
                    start=(kc == 0), stop=(kc == KC - 1))
        rsb_qh = []
        for h2 in range(2):
            dsb = sub.tile([1, NF], FP32, tag="dsb", bufs=4,
                           name=f"dsb_{tag}{t4}_{qh}_{h2}")
            nc.vector.tensor_copy(dsb[:], dens[32 * h2:32 * h2 + 1, :])
            rsb = sub.tile([1, NF], FP32, tag="rsb", bufs=4,
                           name=f"rsb_{tag}{t4}_{qh}_{h2}")
            nc.vector.reciprocal_approx_fast(rsb[:], dsb[:])
            rsb_qh.append(rsb)
        rsbs.append(rsb_qh)
        if dbg is not None and t4 == 0 and tag == "x":
            osb = sub.tile([PC, NF], FP32, tag="osb", bufs=1,
                           name=f"osb{qh}")
            nc.vector.tensor_copy(osb[:], ops[:])
            nc.sync.dma_start(dbg[f"dO{qh}"].ap(), osb[:])
            nc.sync.dma_start(dbg[f"dden{qh}"].ap()[0:1], rsb_qh[0][:])
            nc.sync.dma_start(dbg[f"dden{qh}"].ap()[32:33], rsb_qh[1][:])
            nc.sync.dma_start(dbg[f"dpt{qh}"].ap(), all_pts[qh][0][:])
    dst4 = xT.rearrange("p (j t wb) -> p j t wb", j=8, t=64, wb=8)
    for qh in range(2):
        for h2 in range(2):
            hloc = 2 * t4 + h2
            rb = sub.tile([Dp, NF], FP32, tag="rb", bufs=3,
                          name=f"rb_{tag}{t4}_{qh}_{h2}")
            nc.gpsimd.partition_broadcast(rb[:], rsbs[qh][h2][:])
            toff = 8 * hloc + 4 * qh
            for mm in range(2):
                dst = dst4[Dp * mm:Dp * mm + Dp, :, toff:toff + 4, :]
                src = o_tiles[qh][Dp * h2:Dp * h2 + Dp, :].rearrange(
                    "d (wa wb j m) -> d m j wa wb",
                    wa=4, wb=8, j=8, m=2)[:, mm]
                srb = rb[:].rearrange(
                    "d (wa wb j m) -> d m j wa wb",
                    wa=4, wb=8, j=8, m=2)[:, mm]
                nc.vector.tensor_tensor(dst, src, srb, op=ALU.mult)
    if after_band is not None:
        after_band(t4, o_tiles[1])


def _band_stats(nc, tc, sub, xT, t4, otile, stsb, c, tag):
    """LN stats (sum, sum-sq; 1/D-scaled) of xT band [128*t4, 128*(t4+1))
    accumulated into spare rows 0/32 of the drained AV psum tile, then
    copied into stsb[:, 128*t4 : ...]."""
    for j in range(KC):
        xj = xT[:, j * NF + PC * t4:j * NF + PC * t4 + PC]
        nc.tensor.matmul(otile[0:1, 0:PC], c["onesd"][:], xj,
                         start=(j == 0), stop=(j == KC - 1))
        sq = sub.tile([PC, PC], BF16, tag="sq", bufs=4,
                      name=f"sq_{tag}{t4}_{j}")
        nc.vector.tensor_tensor(sq[:], xj, xj, op=ALU.mult)
        nc.tensor.matmul(otile[32:33, 0:PC], c["onesd"][:], sq[:],
                         start=(j == 0), stop=(j == KC - 1))
    nc.vector.tensor_copy(stsb[:, PC * t4:PC * t4 + PC], otile[0:33, 0:PC])


def _ln_tail(nc, sub, stsb, c, tag):
    """Scalar chain of the row LayerNorm from band stats; returns the
    [128, 512] broadcasts (mub, rstdb)."""
    # preload the sqrt table while upstream drains
    wsq = sub.tile([1, 8], FP32, tag="lrow", bufs=8, name=f"wsq_{tag}")
    nc.gpsimd.memset(wsq[:], 1.0)
    nc.scalar.activation(wsq[:], wsq[:], AF.Sqrt)
    mu2 = sub.tile([1, NF], FP32, tag="lrow", bufs=8, name=f"mu2_{tag}")
    nc.vector.tensor_tensor(mu2[:], stsb[0:1, :], stsb[0:1, :], op=ALU.mult)
    s2r = sub.tile([1, NF], FP32, tag="lrow", bufs=8, name=f"s2r_{tag}")
    nc.vector.tensor_copy(s2r[:], stsb[32:33, :])
    var = sub.tile([1, NF], FP32, tag="lrow", bufs=8, name=f"var_{tag}")
    nc.vector.tensor_tensor(var[:], s2r[:], mu2[:], op=ALU.subtract)
    std = sub.tile([1, NF], FP32, tag="lrow", bufs=8, name=f"std_{tag}")
    nc.scalar.activation(std[:], var[:], AF.Sqrt, bias=c["eps_sc"][:])
    rstd = sub.tile([1, NF], FP32, tag="lrow", bufs=8, name=f"rstd_{tag}")
    nc.vector.reciprocal_approx_fast(rstd[:], std[:])
    mub = sub.tile([PC, NF], FP32, tag="lnb", bufs=2, name=f"mub_{tag}")
    nc.gpsimd.partition_broadcast(mub[:], stsb[0:1, :])
    rstdb = sub.tile([PC, NF], FP32, tag="lnb", bufs=2, name=f"rsb2_{tag}")
    nc.gpsimd.partition_broadcast(rstdb[:], rstd[:])
    return mub, rstdb


def _ln_apply(nc, sub, xT, mub, rstdb, c, tag, write_out):
    for j in range(KC):
        xj = xT[:, j * NF:(j + 1) * NF]
        t1 = sub.tile([PC, NF], BF16, tag="lntmp", bufs=3,
                      name=f"lt_{tag}{j}")
        nc.vector.tensor_tensor(t1[:], xj, mub[:], op=ALU.subtract)
        t2 = sub.tile([PC, NF], BF16, tag="lntmp2", bufs=3,
                      name=f"l2_{tag}{j}")
        nc.vector.tensor_tensor(t2[:], t1[:], rstdb[:], op=ALU.mult)
        write_out(j, t2)


def _emit(nc, tc, dram, ag_in, ag_out, y_out, dbg=None):
    with tc.tile_pool(name="persist", bufs=1) as pp:
        def bias_tile(name):
            shp = list(dram[name].shape)
            return pp.tile(shp, FP32, tag=f"bt_{name}", name=f"bt_{name}")

        bias_names = ("bqc", "bkc", "bq2c", "bk2c", "b1c", "b2c",
                      "gammac", "betac")
        c = {}
        for nm in bias_names:
            c[nm] = bias_tile(nm)
        bvr = bias_tile("bvr")
        bv2r = bias_tile("bv2r")

        def load_biases():
            for nm in bias_names:
                nc.sync.dma_start(c[nm][:], dram[nm].ap())
            nc.sync.dma_start(bvr[:], dram["bvr"].ap())
            nc.sync.dma_start(bv2r[:], dram["bv2r"].ap())

        onesd = pp.tile([PC, 1], BF16, tag="onesd")
        nc.gpsimd.memset(onesd[:], 1.0 / D)
        c["onesd"] = onesd
        onesk = pp.tile([PC, 1], BF16, tag="onesk")
        nc.gpsimd.memset(onesk[:], 1.0)
        c["onesk"] = onesk
        eps_sc = pp.tile([1, 1], FP32, tag="eps_sc")
        nc.gpsimd.memset(eps_sc[:], EPS)
        c["eps_sc"] = eps_sc

        bvB = pp.tile([PC, NF], FP32, tag="bvB")
        bv2B = pp.tile([PC, NF], FP32, tag="bv2B")

        # table warm-up: preload the exp set during initial DMAs
        warm = pp.tile([1, 8], FP32, tag="warm")
        nc.gpsimd.memset(warm[:], 1.0)
        nc.scalar.activation(warm[:], warm[:], AF.Exp)
        # PE warm-up while the first input tiles stream in
        wmm = pp.tile([PC, NF], BF16, tag="wmm")
        nc.gpsimd.memset(wmm[:], 0.0)
        with tc.tile_pool(name="warmps", space="PSUM", bufs=1) as wps:
            wp = wps.tile([1, NF], FP32, tag="warmp", bufs=1)
            for i in range(40):
                nc.tensor.matmul(wp[:], onesd[:], wmm[:],
                                 start=(i == 0), stop=(i == 39))

        # cross-stage persistents
        nTo = pp.tile([PC, KC * NF], BF16, tag="nTo")
        n3T = pp.tile([PC, KC * NF], BF16, tag="n3T")

        # ---- stage 1 ----
        with tc.tile_pool(name="st1", bufs=1) as sub:
            x2own = sub.tile([PC, KC * NF], BF16, tag="x2own")
            qT = [sub.tile([PC, N], BF16, tag="qT", bufs=4, name=f"qT{i}")
                  for i in range(4)]
            kT = [sub.tile([PC, N], BF16, tag="kT", bufs=4, name=f"kT{i}")
                  for i in range(4)]
            vt = [sub.tile([PC, NF], BF16, tag="vt", bufs=KC,
                           name=f"vt{i}") for i in range(KC)]
            xT = sub.tile([PC, KC * NF], BF16, tag="xT")
            stsb1 = sub.tile([33, NF], FP32, tag="lnst", name="stsb1")

            def after_band1(t4, otile):
                bnd = xT[:].rearrange("p (j r) -> p j r", j=KC)[
                    :, :, PC * t4:PC * t4 + PC]
                x2b = x2own[:].rearrange("p (j r) -> p j r", j=KC)[
                    :, :, PC * t4:PC * t4 + PC]
                nc.vector.tensor_tensor(bnd, bnd, x2b, op=ALU.add)
                _band_stats(nc, tc, sub, xT[:], t4, otile, stsb1, c, "b1")

            with tc.tile_pool(name="s1x", bufs=1) as subx:
                x2T = [subx.tile([PC, N], BF16, tag="x2T", bufs=KC,
                                 name=f"x2T{i}") for i in range(KC)]
                for j in range(KC):
                    nc.sync.dma_start(
                        x2T[j][:], dram["x2t"].ap()[j * PC:(j + 1) * PC])
                load_biases()
                nc.gpsimd.partition_broadcast(bvB[:], bvr[:])
                nc.gpsimd.partition_broadcast(bv2B[:], bv2r[:])
                wqts = []
                for m in range(4):
                    wt = subx.tile([PC, KC, PC], BF16, tag="w_q", bufs=4,
                                   name=f"w_q{m}")
                    nc.sync.dma_start(wt[:], dram["wq"].ap()[m])
                    wqts.append(wt)
                x1T = [subx.tile([PC, N], BF16, tag="x1T", bufs=KC,
                                 name=f"x1T{i}") for i in range(KC)]
                for j in range(KC):
                    nc.sync.dma_start(
                        x1T[j][:], dram["x1t"].ap()[j * PC:(j + 1) * PC])
                wkts = []
                for m in range(4):
                    wt = subx.tile([PC, KC, PC], BF16, tag="w_k", bufs=4,
                                   name=f"w_k{m}")
                    nc.sync.dma_start(wt[:], dram["wk"].ap()[m])
                    wkts.append(wt)
                nc.sync.dma_start(x2own[:], dram["x2own"].ap())
                wvts = []
                for kc in range(KC):
                    wt = subx.tile([PC, NF], BF16, tag="wv_v1", bufs=KC,
                                   name=f"wv_v1{kc}")
                    nc.sync.dma_start(wt[:], dram["wv"].ap()[kc])
                    wvts.append(wt)

                with tc.tile_pool(name="s1p", space="PSUM", bufs=1) as psp:
                    q_of = lambda kc, nf: x2T[kc][:, nf * NF:(nf + 1) * NF]
                    k_of = lambda kc, nf: x1T[kc][:, nf * NF:(nf + 1) * NF]
                    all_pts = {}
                    # interleaved emission: per m, project q/k then emit
                    # the score group it unlocks (exp starts early)
                    for m in range(4):
                        _proj_T(nc, subx, psp, dram["wq"], c["bqc"], q_of,
                                qT, "q", m_range=(m, m + 1), wtiles=wqts)
                        _proj_T(nc, subx, psp, dram["wk"], c["bkc"], k_of,
                                kT, "k", m_range=(m, m + 1), wtiles=wkts)
                        if m < 2:
                            for qh in range(2):
                                all_pts[(m, qh)] = _attn_scores(
                                    nc, sub, psp, qT, kT, m, qh, "x")
                        elif m == 2:
                            _proj_v(nc, subx, psp, wvts, bvB,
                                    lambda kc, pc: x1T[kc][
                                        :, pc * PC:(pc + 1) * PC],
                                    vt, "v1", pc_range=(0, 4))
                        else:
                            _proj_v(nc, subx, psp, wvts, bvB,
                                    lambda kc, pc: x1T[kc][
                                        :, pc * PC:(pc + 1) * PC],
                                    vt, "v1", pc_range=(4, 8))
                    for t4 in range(2, 4):
                        for qh in range(2):
                            all_pts[(t4, qh)] = _attn_scores(
                                nc, sub, psp, qT, kT, t4, qh, "x")
                    for t4 in range(4):
                        _attn_group(nc, sub, psp, vt,
                                    [all_pts[(t4, 0)], all_pts[(t4, 1)]],
                                    xT[:], t4, c, "x",
                                    after_band=after_band1,
                                    dbg=dbg if dbg else None)

            mub1, rstdb1 = _ln_tail(nc, sub, stsb1, c, "ln1")

            def ln1_out(j, t2):
                nc.scalar.activation(
                    nTo[:, j * NF:(j + 1) * NF], t2[:], AF.Identity,
                    bias=c["betac"][:, j:j + 1],
                    scale=c["gammac"][:, j:j + 1])
                nc.sync.dma_start(ag_in.ap()[j * PC:(j + 1) * PC],
                                  nTo[:, j * NF:(j + 1) * NF])

            _ln_apply(nc, sub, xT[:], mub1, rstdb1, c, "ln1", ln1_out)
            if dbg:
                nc.sync.dma_start(dbg["dxT"].ap(), xT[:])
                nc.sync.dma_start(dbg["dnTo"].ap(), nTo[:])
            nc.gpsimd.collective_compute(
                "AllGather", ALU.bypass,
                replica_groups=[[0, 1], [2, 3], [4, 5], [6, 7]],
                ins=[ag_in.ap()], outs=[ag_out.ap()])

        w1pre = [pp.tile([PC, KC, PC], BF16, tag="w1pre", bufs=8,
                         name=f"w1pre{i}") for i in range(8)]

        # ---- stage 2 ----
        # keys are used in arrival order [own rows | partner rows]
        # (softmax is key-permutation invariant); queries need global
        # order, which nTg (both gathered blocks) provides uniformly.
        with tc.tile_pool(name="st2", bufs=1) as sub:
            for f in range(8):
                nc.sync.dma_start(w1pre[f][:], dram["w1"].ap()[f])
            q2T = [sub.tile([PC, N], BF16, tag="q2T", bufs=4,
                            name=f"q2T{i}") for i in range(4)]
            k2T = [sub.tile([PC, N], BF16, tag="k2T", bufs=4,
                            name=f"k2T{i}") for i in range(4)]
            v2t = [sub.tile([PC, NF], BF16, tag="v2t", bufs=KC,
                            name=f"v2t{i}") for i in range(KC)]
            x3T = sub.tile([PC, KC * NF], BF16, tag="x3T")
            stsb2 = sub.tile([33, NF], FP32, tag="lnst", name="stsb2")

            def after_band2(t4, otile):
                bnd = x3T[:].rearrange("p (j r) -> p j r", j=KC)[
                    :, :, PC * t4:PC * t4 + PC]
                nob = nTo[:].rearrange("p (j r) -> p j r", j=KC)[
                    :, :, PC * t4:PC * t4 + PC]
                nc.vector.tensor_tensor(bnd, bnd, nob, op=ALU.add)
                _band_stats(nc, tc, sub, x3T[:], t4, otile, stsb2, c, "b2")

            wv2ts = []
            for kc in range(KC):
                wt = sub.tile([PC, NF], BF16, tag="wv_v2", bufs=KC,
                              name=f"wv_v2{kc}")
                nc.sync.dma_start(wt[:], dram["wv2"].ap()[kc])
                wv2ts.append(wt)

            with tc.tile_pool(name="s2p", space="PSUM", bufs=1) as psp:
                # own-row halves of k2/v2 run from nTo while the
                # AllGather is in flight
                _proj_v(nc, sub, psp, wv2ts, bv2B,
                        lambda kc, pc: nTo[:, kc * NF + pc * PC:
                                           kc * NF + (pc + 1) * PC],
                        v2t, "v2o", pc_range=(0, 4))
                k2w = [sub.tile([PC, KC, PC], BF16, tag="w_k2", bufs=4,
                                name=f"wk2_{m}") for m in range(4)]
                for m in range(4):
                    nc.sync.dma_start(k2w[m][:], dram["wk2"].ap()[m])
                for m in range(4):
                    ps = psp.tile([PC, NF], FP32, tag="proj", bufs=2,
                                  name=f"k2o_{m}")
                    for kc in range(KC):
                        nc.tensor.matmul(
                            ps[:], k2w[m][:, kc, :],
                            nTo[:, kc * NF:kc * NF + NF],
                            start=(kc == 0), stop=(kc == KC - 1))
                    nc.scalar.activation(k2T[m][:, 0:NF], ps[:],
                                         AF.Identity,
                                         bias=c["bk2c"][:, m:m + 1])

                # keep PE warm through the gather window
                wp2 = psp.tile([1, NF], FP32, tag="proj", bufs=2,
                               name="cwarm")
                for i in range(64):
                    nc.tensor.matmul(wp2[:], c["onesd"][:], k2T[0][:, 0:NF],
                                     start=(i == 0), stop=(i == 63))

                # gathered blocks (global row order) + exact partner
                # recovery: partner = (block0 - own) + block1
                nTg = [sub.tile([PC, N], BF16, tag="nTg", bufs=KC,
                                name=f"nTg{i}") for i in range(KC)]
                for j in range(KC):
                    for r in range(2):
                        nc.sync.dma_start(
                            nTg[j][:, r * NF:(r + 1) * NF],
                            ag_out.ap()[r, j * PC:(j + 1) * PC])
                nTp = [sub.tile([PC, NF], BF16, tag="nTp", bufs=KC,
                                name=f"nTp{i}") for i in range(KC)]
                for j in range(KC):
                    tdif = sub.tile([PC, NF], FP32, tag="tdif", bufs=4,
                                    name=f"tdif{j}")
                    nc.vector.tensor_tensor(
                        tdif[:], nTg[j][:, 0:NF],
                        nTo[:, j * NF:(j + 1) * NF], op=ALU.subtract)
                    nc.vector.tensor_tensor(
                        nTp[j][:], tdif[:], nTg[j][:, NF:N], op=ALU.add)

                # rest of the MLP fc1 weights stream in the background
                for f in range(8, FT):
                    pass  # prefetched in stage 3 (SBUF budget)

                q2_of = lambda kc, nf: nTg[kc][:, nf * NF:(nf + 1) * NF]
                all_pts = {}
                for m in range(4):
                    # partner half of k2 for this m
                    ps = psp.tile([PC, NF], FP32, tag="proj", bufs=2,
                                  name=f"k2p_{m}")
                    for kc in range(KC):
                        nc.tensor.matmul(
                            ps[:], k2w[m][:, kc, :], nTp[kc][:],
                            start=(kc == 0), stop=(kc == KC - 1))
                    nc.scalar.activation(k2T[m][:, NF:N], ps[:],
                                         AF.Identity,
                                         bias=c["bk2c"][:, m:m + 1])
                    _proj_T(nc, sub, psp, dram["wq2"], c["bq2c"], q2_of,
                            q2T, "q2", m_range=(m, m + 1))
                    if m < 2:
                        for qh in range(2):
                            all_pts[(m, qh)] = _attn_scores(
                                nc, sub, psp, q2T, k2T, m, qh, "y")
                    elif m == 2:
                        _proj_v(nc, sub, psp, wv2ts, bv2B,
                                lambda kc, pc: nTp[kc][
                                    :, (pc - 4) * PC:(pc - 3) * PC],
                                v2t, "v2p", pc_range=(4, 8))
                for t4 in range(2, 4):
                    for qh in range(2):
                        all_pts[(t4, qh)] = _attn_scores(
                            nc, sub, psp, q2T, k2T, t4, qh, "y")
                for t4 in range(4):
                    _attn_group(nc, sub, psp, v2t,
                                [all_pts[(t4, 0)], all_pts[(t4, 1)]],
                                x3T[:], t4, c, "y",
                                after_band=after_band2)

            mub2, rstdb2 = _ln_tail(nc, sub, stsb2, c, "ln2")

            def ln2_out(j, t2):
                nc.scalar.activation(
                    n3T[:, j * NF:(j + 1) * NF], t2[:], AF.Identity,
                    bias=c["betac"][:, j:j + 1],
                    scale=c["gammac"][:, j:j + 1])

            _ln_apply(nc, sub, x3T[:], mub2, rstdb2, c, "ln2", ln2_out)
            if dbg:
                nc.sync.dma_start(dbg["dx3T"].ap(), x3T[:])
                nc.sync.dma_start(dbg["dn3T"].ap(), n3T[:])

        # ---- stage 3: MLP ----
        with tc.tile_pool(name="s3", bufs=1) as sub:
            hT = [sub.tile([PC, NF], BF16, tag="hT", bufs=FT,
                           name=f"hT{i}") for i in range(FT)]
            w2pre = [sub.tile([PC, FT, PC], BF16, tag="w2t", bufs=8,
                              name=f"w2t{d}") for d in range(KC)]
            with tc.tile_pool(name="s3p", space="PSUM", bufs=1) as psp:
                for f in range(FT):
                    if f < 8:
                        wt = w1pre[f]
                    else:
                        wt = sub.tile([PC, KC, PC], BF16, tag="w1t", bufs=6,
                                      name=f"w1t{f}")
                        nc.sync.dma_start(wt[:], dram["w1"].ap()[f])
                    if f < KC:
                        nc.sync.dma_start(w2pre[f][:], dram["w2"].ap()[f])
                    ps = psp.tile([PC, NF], FP32, tag="mlp", bufs=8,
                                  name=f"h{f}")
                    for kc in range(KC):
                        nc.tensor.matmul(
                            ps[:], wt[:, kc, :],
                            n3T[:, kc * NF:(kc + 1) * NF],
                            start=(kc == 0), stop=(kc == KC - 1))
                    nc.scalar.activation(hT[f][:], ps[:], AF.Gelu,
                                         bias=c["b1c"][:, f:f + 1])
                for d in range(KC):
                    ps = psp.tile([PC, NF], FP32, tag="mlp", bufs=8,
                                  name=f"yp{d}")
                    for f in range(FT):
                        nc.tensor.matmul(ps[:], w2pre[d][:, f, :], hT[f][:],
                                         start=(f == 0), stop=(f == FT - 1))
                    yt = sub.tile([PC, NF], FP32, tag="yT", bufs=4,
                                  name=f"yT{d}")
                    nc.vector.scalar_tensor_tensor(
                        yt[:], ps[:], c["b2c"][:, d:d + 1],
                        n3T[:, d * NF:(d + 1) * NF],
                        op0=ALU.add, op1=ALU.add)
                    nc.sync.dma_start(
                        y_out.ap()[d * PC:(d + 1) * PC], yt[:])


def _get_nc():
    if "nc" not in _CACHE:
        _CACHE["nc"] = _build()
    return _CACHE["nc"]


def _prep_inputs(inputs):
    """Host-side slicing/transposition into per-core bf16 DRAM layouts."""
    f32 = np.float32
    x1 = np.ascontiguousarray(np.asarray(inputs["x1"], f32))
    x2 = np.ascontiguousarray(np.asarray(inputs["x2"], f32))
    Wq = np.asarray(inputs["Wq"], f32)
    Wkv = np.asarray(inputs["Wkv"], f32)
    Wqkv = np.asarray(inputs["Wqkv"], f32)
    W1 = np.asarray(inputs["W1"], f32)
    W2 = np.asarray(inputs["W2"], f32)
    bq = np.asarray(inputs["bq"], f32)
    bkv = np.asarray(inputs["bkv"], f32)
    bqkv = np.asarray(inputs["bqkv"], f32)
    gamma = np.asarray(inputs["gamma"], f32)
    beta = np.asarray(inputs["beta"], f32)
    b1 = np.asarray(inputs["b1"], f32)
    b2 = np.asarray(inputs["b2"], f32)

    def wcols(Wslice):     # (1024, 512) -> (4, 128, 8, 128) bf16
        return np.ascontiguousarray(
            Wslice.reshape(KC, PC, 4, PC).transpose(2, 1, 0, 3)).astype(BF)

    def bcols(bslice, n):  # (n*128,) -> (128, n) fp32
        return np.ascontiguousarray(bslice.reshape(n, PC).T)

    w1h = np.ascontiguousarray(
        W1.reshape(KC, PC, FT, PC).transpose(2, 1, 0, 3)).astype(BF)
    w2h = np.ascontiguousarray(
        W2.reshape(FT, PC, KC, PC).transpose(2, 1, 0, 3)).astype(BF)
    b1h = bcols(b1, FT)
    b2h = bcols(b2, KC)
    gh = bcols(gamma, KC)
    bh = bcols(beta, KC)

    in_maps = []
    for core in range(8):
        b, hh = core // 2, core % 2
        lo = NF * hh
        x2t = np.ascontiguousarray(x2[b].T)
        x1t = np.ascontiguousarray(x1[b].T)
        x2own = np.ascontiguousarray(
            x2t[:, lo:lo + NF].reshape(KC, PC, NF).transpose(1, 0, 2)
            .reshape(PC, KC * NF)).astype(BF)
        in_maps.append({
            "x2t": x2t.astype(BF), "x1t": x1t.astype(BF), "x2own": x2own,
            "wq": wcols(Wq[:, lo:lo + NF]),
            "wk": wcols(Wkv[:, lo:lo + NF]),
            "wv": np.ascontiguousarray(
                Wkv[:, D + lo:D + lo + NF].reshape(KC, PC, NF)).astype(BF),
            "wq2": wcols(Wqkv[:, lo:lo + NF]),
            "wk2": wcols(Wqkv[:, D + lo:D + lo + NF]),
            "wv2": np.ascontiguousarray(
                Wqkv[:, 2 * D + lo:2 * D + lo + NF]
                .reshape(KC, PC, NF)).astype(BF),
            "w1": w1h, "w2": w2h,
            "bqc": bcols(bq[lo:lo + NF], 4),
            "bkc": bcols(bkv[lo:lo + NF], 4),
            "bq2c": bcols(bqkv[lo:lo + NF], 4),
            "bk2c": bcols(bqkv[D + lo:D + lo + NF], 4),
            "bvr": np.ascontiguousarray(
                bkv[D + lo:D + lo + NF].reshape(1, NF)),
            "bv2r": np.ascontiguousarray(
                bqkv[2 * D + lo:2 * D + lo + NF].reshape(1, NF)),
            "b1c": b1h, "b2c": b2h, "gammac": gh, "betac": bh,
        })
    return in_maps


def kernel(**inputs):
    in_maps = _prep_inputs(inputs)
    nc = _get_nc()
    res = run_bass_kernel_spmd(nc, in_maps, core_ids=list(range(8)))
    _CACHE["last_results"] = res
    out = np.zeros((B, N, D), np.float32)
    for core in range(8):
        b, hh = core // 2, core % 2
        out[b, NF * hh:NF * hh + NF, :] = res.results[core]["y"].T
    return out


# revision 17
# speedup vs baseline: 1.0344x; 1.0344x over previous
"""Trainium2 Bass kernel for nn_CrossSelfDecoder (B=4,N=1024,D=1024,H=16,F=4096).

Sharding: 8 cores = (batch b in 0..3) x (head-half hh in 0..1). Each core
computes attention for its 8 heads over all 1024 positions of its batch.
Because the reference reshapes (B,H,N,Dp)->(B,N,D) without permuting heads
back, head-ownership makes row-ownership invariant: core (b,hh) owns rows
[512*hh, 512*hh+512) of batch b through the whole network.

Design (v2):
- Host pre-transposes x1/x2 and pre-tiles all weights into bf16 DRAM
  layouts: zero device-side transposes, contiguous per-partition DMAs.
- All matmuls bf16 x bf16 with fp32 PSUM accumulate.
- Attention processed per head-PAIR (t4 = hloc//2): scores of the even/odd
  head are row-packed (K=64 tiles at array rows 0-63 / 64-127, concurrent);
  AV of the pair is column-packed into one [128,512] PSUM bank (M=64 tiles
  at array cols 0-63 / 64-127); softmax denominators are 4x column-packed
  M=1 matmuls (ones lhsT) deferred to the end of the pair's group.
- Projections are emitted interleaved with score groups so the Scalar
  engine's EXP stream (the attention pacer) starts ~40us early.
- LayerNorm statistics accumulate per 128-row band as attention groups
  complete (into spare rows of the drained AV PSUM bank); only the small
  scalar chain + apply remain as a tail.
- One pairwise AllGather of the LN1 rows (bf16) overlapped with the
  own-row halves of the k2/v2 projections; partner rows recovered as
  (block0 - own) + block1.
"""

import numpy as np
import ml_dtypes

import concourse.mybir as mybir
import concourse.tile as tile
from concourse import bacc
from concourse.bass_utils import run_bass_kernel_spmd

FP32 = mybir.dt.float32
BF16 = mybir.dt.bfloat16
AF = mybir.ActivationFunctionType
ALU = mybir.AluOpType

B, N, D, H, F = 4, 1024, 1024, 16, 4096
Dp = D // H           # 64
HPC = 8               # heads per core
PC = 128              # partition chunk
NF = 512              # free chunk (one psum bank of fp32)
KC = D // PC          # 8 contraction chunks
FT = F // PC          # 32 f-tiles
EPS = 1e-5
BF = ml_dtypes.bfloat16

_CACHE = {}


def _build():
    nc = bacc.Bacc("TRN2", target_bir_lowering=False, debug=False,
                   num_devices=8)
    dram = {}
    specs = [
        ("x2t", [D, N], BF16), ("x1t", [D, N], BF16),
        ("x2own", [PC, KC * NF], BF16),
        ("wq", [4, PC, KC, PC], BF16), ("wk", [4, PC, KC, PC], BF16),
        ("wv", [KC, PC, NF], BF16),
        ("wq2", [4, PC, KC, PC], BF16), ("wk2", [4, PC, KC, PC], BF16),
        ("wv2", [KC, PC, NF], BF16),
        ("w1", [FT, PC, KC, PC], BF16), ("w2", [KC, PC, FT, PC], BF16),
        ("bqc", [PC, 4], FP32), ("bkc", [PC, 4], FP32),
        ("bq2c", [PC, 4], FP32), ("bk2c", [PC, 4], FP32),
        ("bvr", [1, NF], FP32), ("bv2r", [1, NF], FP32),
        ("b1c", [PC, FT], FP32), ("b2c", [PC, KC], FP32),
        ("gammac", [PC, KC], FP32), ("betac", [PC, KC], FP32),
    ]
    for nm, shp, dt in specs:
        dram[nm] = nc.dram_tensor(nm, shp, dt, kind="ExternalInput")
    y_out = nc.dram_tensor("y", [D, NF], FP32, kind="ExternalOutput")
    dbg = {}
    if _CACHE.get("debug"):
        for nm in ("dxT", "dnTo", "dx3T", "dn3T"):
            dbg[nm] = nc.dram_tensor(nm, [PC, KC * NF], BF16,
                                     kind="ExternalOutput")
        for qh in range(2):
            dbg[f"dO{qh}"] = nc.dram_tensor(f"dO{qh}", [PC, NF], FP32,
                                            kind="ExternalOutput")
            dbg[f"dden{qh}"] = nc.dram_tensor(f"dden{qh}", [33, NF], FP32,
                                              kind="ExternalOutput")
            dbg[f"dpt{qh}"] = nc.dram_tensor(f"dpt{qh}", [PC, 2 * NF], BF16,
                                             kind="ExternalOutput")

    # collective staging: own LN1 rows out, both group blocks back
    ag_in = nc.dram_tensor("agin", [D, NF], BF16, kind="Internal")
    ag_out = nc.dram_tensor("agout", [2, D, NF], BF16, kind="Internal")

    with tile.TileContext(nc) as tc:
        _emit(nc, tc, dram, ag_in, ag_out, y_out, dbg)
    nc.compile()
    return nc


def _proj_T(nc, sub, psp, w_dram, bias_cols, rhs_of, out_tiles, tag,
            nf_range=(0, 2), m_range=(0, 4), wtiles=None):
    """out[m][:, nf*512:...] = (W.T @ rhs + bias), T-domain."""
    for m in range(*m_range):
        if wtiles is not None:
            wt = wtiles[m]
        else:
            wt = sub.tile([PC, KC, PC], BF16, tag=f"w_{tag}", bufs=2,
                          name=f"w_{tag}{m}")
            nc.sync.dma_start(wt[:], w_dram.ap()[m])
        for nf in range(*nf_range):
            ps = psp.tile([PC, NF], FP32, tag="proj", bufs=2,
                          name=f"proj_{tag}{m}_{nf}")
            for kc in range(KC):
                nc.tensor.matmul(ps[:], wt[:, kc, :], rhs_of(kc, nf),
                                 start=(kc == 0), stop=(kc == KC - 1))
            nc.scalar.activation(out_tiles[m][:, nf * NF:(nf + 1) * NF],
                                 ps[:], AF.Identity,
                                 bias=bias_cols[:, m:m + 1])


def _proj_v(nc, sub, psp, wts, bvB, rhs_of, v_tiles, tag, pc_range=(0, 8)):
    """v natural: v_tiles[pc] [128 keys, 512 own-head cols]."""
    for pc in range(*pc_range):
        ps = psp.tile([PC, NF], FP32, tag="proj", bufs=2, name=f"v_{tag}{pc}")
        for kc in range(KC):
            nc.tensor.matmul(ps[:], rhs_of(kc, pc), wts[kc][:],
                             start=(kc == 0), stop=(kc == KC - 1))
        nc.vector.tensor_tensor(v_tiles[pc][:], ps[:], bvB[:], op=ALU.add)


def _attn_scores(nc, sub, psp, qT, kT, t4, qh, tag):
    """Row-packed scores + exp for head pair (2t4, 2t4+1), query half qh.
    Returns the 8 pt tiles [128, 2*NF] (free: h2-half x 512 queries)."""
    pts = []
    for kc in range(KC):
        sps = psp.tile([PC, 2 * NF], FP32, tag="S", bufs=2,
                       name=f"S_{tag}{t4}_{qh}_{kc}")
        for h2 in range(2):
            r64 = Dp * h2
            nc.tensor.matmul(
                sps[:, h2 * NF:(h2 + 1) * NF],
                kT[t4][r64:r64 + Dp, kc * PC:(kc + 1) * PC],
                qT[t4][r64:r64 + Dp, qh * NF:(qh + 1) * NF],
                start=True, stop=True)
        pt = sub.tile([PC, 2 * NF], BF16, tag="PT", bufs=14,
                      name=f"PT_{tag}{t4}_{qh}_{kc}")
        nc.scalar.activation(pt[:], sps[:], AF.Exp)
        pts.append(pt)
    return pts


def _attn_group(nc, sub, psp, v_tiles, all_pts, xT, t4, c, tag,
                after_band=None, dbg=None):
    """AV (col-packed pairs) + dens (4x col-packed) + normalize + scatter
    for head pair t4 (both query halves). all_pts[qh] = pt tiles."""
    o_tiles = []
    rsbs = []
    for qh in range(2):
        ops = psp.tile([PC, NF], FP32, tag="O", bufs=2,
                       name=f"O_{tag}{t4}_{qh}")
        dens = psp.tile([33, NF], FP32, tag="proj", bufs=2,
                        name=f"den_{tag}{t4}_{qh}")
        for kc in range(KC):
            for h2 in range(2):
                nc.tensor.matmul(
                    ops[Dp * h2:Dp * (h2 + 1), :],
                    v_tiles[kc][:, Dp * (2 * t4 + h2):
                                Dp * (2 * t4 + h2 + 1)],
                    all_pts[qh][kc][:, h2 * NF:(h2 + 1) * NF],
                    start=(kc == 0), stop=(kc == KC - 1))
            for h2 in range(2):
                nc.tensor.matmul(
                    dens[32 * h2:32 * h2 + 1, :], c["onesk"][:],
                    all_pts[qh][kc][:, h2 * NF:(h2 + 1) * NF],
                    start=(kc == 0), stop=(kc == KC - 1))
        o_tiles.append(ops)
        rsb_qh = []
        for h2 in range(2):
            dsb = sub.tile([1, NF], FP32, tag="dsb", bufs=4,
                           name=f"dsb_{tag}{t4}_{qh}_{h2}")
            nc.vector.tensor_copy(dsb[:], dens[32 * h2:32 * h2 + 1, :])
            rsb = sub.tile([1, NF], FP32, tag="rsb", bufs=4,
                           name=f"rsb_{tag}{t4}_{qh}_{h2}")
            nc.vector.reciprocal_approx_fast(rsb[:], dsb[:])
            rsb_qh.append(rsb)
        rsbs.append(rsb_qh)
        if dbg is not None and t4 == 0 and tag == "x":
            osb = sub.tile([PC, NF], FP32, tag="osb", bufs=1,
                           name=f"osb{qh}")
            nc.vector.tensor_copy(osb[:], ops[:])
            nc.sync.dma_start(dbg[f"dO{qh}"].ap(), osb[:])
            nc.sync.dma_start(dbg[f"dden{qh}"].ap()[0:1], rsb_qh[0][:])
            nc.sync.dma_start(dbg[f"dden{qh}"].ap()[32:33], rsb_qh[1][:])
            nc.sync.dma_start(dbg[f"dpt{qh}"].ap(), all_pts[qh][0][:])
    dst4 = xT.rearrange("p (j t wb) -> p j t wb", j=8, t=64, wb=8)
    for qh in range(2):
        for h2 in range(2):
            hloc = 2 * t4 + h2
            rb = sub.tile([Dp, NF], FP32, tag="rb", bufs=3,
                          name=f"rb_{tag}{t4}_{qh}_{h2}")
            nc.gpsimd.partition_broadcast(rb[:], rsbs[qh][h2][:])
            toff = 8 * hloc + 4 * qh
            for mm in range(2):
                dst = dst4[Dp * mm:Dp * mm + Dp, :, toff:toff + 4, :]
                src = o_tiles[qh][Dp * h2:Dp * h2 + Dp, :].rearrange(
                    "d (wa wb j m) -> d m j wa wb",
                    wa=4, wb=8, j=8, m=2)[:, mm]
                srb = rb[:].rearrange(
                    "d (wa wb j m) -> d m j wa wb",
                    wa=4, wb=8, j=8, m=2)[:, mm]
                nc.vector.tensor_tensor(dst, src, srb, op=ALU.mult)
    if after_band is not None:
        after_band(t4, o_tiles[1])


def _band_stats(nc, tc, sub, xT, t4, otile, stsb, c, tag):
    """LN stats (sum, sum-sq; 1/D-scaled) of xT band [128*t4, 128*(t4+1))
    accumulated into spare rows 0/32 of the drained AV psum tile, then
    copied into stsb[:, 128*t4 : ...]."""
    for j in range(KC):
        xj = xT[:, j * NF + PC * t4:j * NF + PC * t4 + PC]
        nc.tensor.matmul(otile[0:1, 0:PC], c["onesd"][:], xj,
                         start=(j == 0), stop=(j == KC - 1))
        sq = sub.tile([PC, PC], BF16, tag="sq", bufs=4,
                      name=f"sq_{tag}{t4}_{j}")
        nc.vector.tensor_tensor(sq[:], xj, xj, op=ALU.mult)
        nc.tensor.matmul(otile[32:33, 0:PC], c["onesd"][:], sq[:],
                         start=(j == 0), stop=(j == KC - 1))
    nc.vector.tensor_copy(stsb[:, PC * t4:PC * t4 + PC], otile[0:33, 0:PC])


def _ln_tail(nc, sub, stsb, c, tag):
    """Scalar chain of the row LayerNorm from band stats; returns the
    [128, 512] broadcasts (mub, rstdb)."""
    # preload the sqrt table while upstream drains
    wsq = sub.tile([1, 8], FP32, tag="lrow", bufs=8, name=f"wsq_{tag}")
    nc.gpsimd.memset(wsq[:], 1.0)
    nc.scalar.activation(wsq[:], wsq[:], AF.Sqrt)
    mu2 = sub.tile([1, NF], FP32, tag="lrow", bufs=8, name=f"mu2_{tag}")
    nc.vector.tensor_tensor(mu2[:], stsb[0:1, :], stsb[0:1, :], op=ALU.mult)
    s2r = sub.tile([1, NF], FP32, tag="lrow", bufs=8, name=f"s2r_{tag}")
    nc.vector.tensor_copy(s2r[:], stsb[32:33, :])
    var = sub.tile([1, NF], FP32, tag="lrow", bufs=8, name=f"var_{tag}")
    nc.vector.tensor_tensor(var[:], s2r[:], mu2[:], op=ALU.subtract)
    std = sub.tile([1, NF], FP32, tag="lrow", bufs=8, name=f"std_{tag}")
    nc.scalar.activation(std[:], var[:], AF.Sqrt, bias=c["eps_sc"][:])
    rstd = sub.tile([1, NF], FP32, tag="lrow", bufs=8, name=f"rstd_{tag}")
    nc.vector.reciprocal_approx_fast(rstd[:], std[:])
    mub = sub.tile([PC, NF], FP32, tag="lnb", bufs=2, name=f"mub_{tag}")
    nc.gpsimd.partition_broadcast(mub[:], stsb[0:1, :])
    rstdb = sub.tile([PC, NF], FP32, tag="lnb", bufs=2, name=f"rsb2_{tag}")
    nc.gpsimd.partition_broadcast(rstdb[:], rstd[:])
    return mub, rstdb


def _ln_apply(nc, sub, xT, mub, rstdb, c, tag, write_out):
    for j in range(KC):
        xj = xT[:, j * NF:(j + 1) * NF]
        t1 = sub.tile([PC, NF], BF16, tag="lntmp", bufs=3,
                      name=f"lt_{tag}{j}")
        nc.vector.tensor_tensor(t1[:], xj, mub[:], op=ALU.subtract)
        t2 = sub.tile([PC, NF], BF16, tag="lntmp2", bufs=3,
                      name=f"l2_{tag}{j}")
        nc.vector.tensor_tensor(t2[:], t1[:], rstdb[:], op=ALU.mult)
        write_out(j, t2)


def _emit(nc, tc, dram, ag_in, ag_out, y_out, dbg=None):
    with tc.tile_pool(name="persist", bufs=1) as pp:
        def bias_tile(name):
            shp = list(dram[name].shape)
            return pp.tile(shp, FP32, tag=f"bt_{name}", name=f"bt_{name}")

        bias_names = ("bqc", "bkc", "bq2c", "bk2c", "b1c", "b2c",
                      "gammac", "betac")
        c = {}
        for nm in bias_names:
            c[nm] = bias_tile(nm)
        bvr = bias_tile("bvr")
        bv2r = bias_tile("bv2r")

        def load_biases():
            for nm in bias_names:
                nc.sync.dma_start(c[nm][:], dram[nm].ap())
            nc.sync.dma_start(bvr[:], dram["bvr"].ap())
            nc.sync.dma_start(bv2r[:], dram["bv2r"].ap())

        onesd = pp.tile([PC, 1], BF16, tag="onesd")
        nc.gpsimd.memset(onesd[:], 1.0 / D)
        c["onesd"] = onesd
        onesk = pp.tile([PC, 1], BF16, tag="onesk")
        nc.gpsimd.memset(onesk[:], 1.0)
        c["onesk"] = onesk
        eps_sc = pp.tile([1, 1], FP32, tag="eps_sc")
        nc.gpsimd.memset(eps_sc[:], EPS)
        c["eps_sc"] = eps_sc

        bvB = pp.tile([PC, NF], FP32, tag="bvB")
        bv2B = pp.tile([PC, NF], FP32, tag="bv2B")

        # table warm-up: preload the exp set during initial DMAs
        warm = pp.tile([1, 8], FP32, tag="warm")
        nc.gpsimd.memset(warm[:], 1.0)
        nc.scalar.activation(warm[:], warm[:], AF.Exp)
        # PE warm-up while the first input tiles stream in
        wmm = pp.tile([PC, NF], BF16, tag="wmm")
        nc.gpsimd.memset(wmm[:], 0.0)
        with tc.tile_pool(name="warmps", space="PSUM", bufs=1) as wps:
            wp = wps.tile([1, NF], FP32, tag="warmp", bufs=1)
            for i in range(40):
                nc.tensor.matmul(wp[:], onesd[:], wmm[:],
                                 start=(i == 0), stop=(i == 39))

        # cross-stage persistents
        nTo = pp.tile([PC, KC * NF], BF16, tag="nTo")
        n3T = pp.tile([PC, KC * NF], BF16, tag="n3T")

        # ---- stage 1 ----
        with tc.tile_pool(name="st1", bufs=1) as sub:
            x2own = sub.tile([PC, KC * NF], BF16, tag="x2own")
            qT = [sub.tile([PC, N], BF16, tag="qT", bufs=4, name=f"qT{i}")
                  for i in range(4)]
            kT = [sub.tile([PC, N], BF16, tag="kT", bufs=4, name=f"kT{i}")
                  for i in range(4)]
            vt = [sub.tile([PC, NF], BF16, tag="vt", bufs=KC,
                           name=f"vt{i}") for i in range(KC)]
            xT = sub.tile([PC, KC * NF], BF16, tag="xT")
            stsb1 = sub.tile([33, NF], FP32, tag="lnst", name="stsb1")

            def after_band1(t4, otile):
                bnd = xT[:].rearrange("p (j r) -> p j r", j=KC)[
                    :, :, PC * t4:PC * t4 + PC]
                x2b = x2own[:].rearrange("p (j r) -> p j r", j=KC)[
                    :, :, PC * t4:PC * t4 + PC]
                nc.vector.tensor_tensor(bnd, bnd, x2b, op=ALU.add)
                _band_stats(nc, tc, sub, xT[:], t4, otile, stsb1, c, "b1")

            with tc.tile_pool(name="s1x", bufs=1) as subx:
                x2T = [subx.tile([PC, N], BF16, tag="x2T", bufs=KC,
                                 name=f"x2T{i}") for i in range(KC)]
                for j in range(KC):
                    nc.sync.dma_start(
                        x2T[j][:], dram["x2t"].ap()[j * PC:(j + 1) * PC])
                load_biases()
                nc.gpsimd.partition_broadcast(bvB[:], bvr[:])
                nc.gpsimd.partition_broadcast(bv2B[:], bv2r[:])
                wqts = []
                for m in range(4):
                    wt = subx.tile([PC, KC, PC], BF16, tag="w_q", bufs=4,
                                   name=f"w_q{m}")
                    nc.sync.dma_start(wt[:], dram["wq"].ap()[m])
                    wqts.append(wt)
                x1T = [subx.tile([PC, N], BF16, tag="x1T", bufs=KC,
                                 name=f"x1T{i}") for i in range(KC)]
                for j in range(KC):
                    nc.sync.dma_start(
                        x1T[j][:], dram["x1t"].ap()[j * PC:(j + 1) * PC])
                wkts = []
                for m in range(4):
                    wt = subx.tile([PC, KC, PC], BF16, tag="w_k", bufs=4,
                                   name=f"w_k{m}")
                    nc.sync.dma_start(wt[:], dram["wk"].ap()[m])
                    wkts.append(wt)
                nc.sync.dma_start(x2own[:], dram["x2own"].ap())
                wvts = []
                for kc in range(KC):
                    wt = subx.tile([PC, NF], BF16, tag="wv_v1", bufs=KC,
                                   name=f"wv_v1{kc}")
                    nc.sync.dma_start(wt[:], dram["wv"].ap()[kc])
                    wvts.append(wt)

                with tc.tile_pool(name="s1p", space="PSUM", bufs=1) as psp:
                    q_of = lambda kc, nf: x2T[kc][:, nf * NF:(nf + 1) * NF]
                    k_of = lambda kc, nf: x1T[kc][:, nf * NF:(nf + 1) * NF]
                    all_pts = {}
                    # interleaved emission: per m, project q/k then emit
                    # the score group it unlocks (exp starts early)
                    for m in range(4):
                        _proj_T(nc, subx, psp, dram["wq"], c["bqc"], q_of,
                                qT, "q", m_range=(m, m + 1), wtiles=wqts)
                        _proj_T(nc, subx, psp, dram["wk"], c["bkc"], k_of,
                                kT, "k", m_range=(m, m + 1), wtiles=wkts)
                        if m < 2:
                            for qh in range(2):
                                all_pts[(m, qh)] = _attn_scores(
                                    nc, sub, psp, qT, kT, m, qh, "x")
                        elif m == 2:
                            _proj_v(nc, subx, psp, wvts, bvB,
                                    lambda kc, pc: x1T[kc][
                                        :, pc * PC:(pc + 1) * PC],
                                    vt, "v1", pc_range=(0, 4))
                        else:
                            _proj_v(nc, subx, psp, wvts, bvB,
                                    lambda kc, pc: x1T[kc][
                                        :, pc * PC:(pc + 1) * PC],
                                    vt, "v1", pc_range=(4, 8))
                    for t4 in range(2, 4):
                        for qh in range(2):
                            all_pts[(t4, qh)] = _attn_scores(
                                nc, sub, psp, qT, kT, t4, qh, "x")
                    for t4 in range(4):
                        _attn_group(nc, sub, psp, vt,
                                    [all_pts[(t4, 0)], all_pts[(t4, 1)]],
                                    xT[:], t4, c, "x",
                                    after_band=after_band1,
                                    dbg=dbg if dbg else None)

            mub1, rstdb1 = _ln_tail(nc, sub, stsb1, c, "ln1")

            def ln1_out(j, t2):
                nc.scalar.activation(
                    nTo[:, j * NF:(j + 1) * NF], t2[:], AF.Identity,
                    bias=c["betac"][:, j:j + 1],
                    scale=c["gammac"][:, j:j + 1])
                nc.sync.dma_start(ag_in.ap()[j * PC:(j + 1) * PC],
                                  nTo[:, j * NF:(j + 1) * NF])

            _ln_apply(nc, sub, xT[:], mub1, rstdb1, c, "ln1", ln1_out)
            if dbg:
                nc.sync.dma_start(dbg["dxT"].ap(), xT[:])
                nc.sync.dma_start(dbg["dnTo"].ap(), nTo[:])
            nc.gpsimd.collective_compute(
                "AllGather", ALU.bypass,
                replica_groups=[[0, 1], [2, 3], [4, 5], [6, 7]],
                ins=[ag_in.ap()], outs=[ag_out.ap()])


        # ---- stage 2 ----
        # keys are used in arrival order [own rows | partner rows]
        # (softmax is key-permutation invariant); queries need global
        # order, which nTg (both gathered blocks) provides uniformly.
        with tc.tile_pool(name="st2", bufs=1) as sub:
            q2T = [sub.tile([PC, N], BF16, tag="q2T", bufs=4,
                            name=f"q2T{i}") for i in range(4)]
            k2T = [sub.tile([PC, N], BF16, tag="k2T", bufs=4,
                            name=f"k2T{i}") for i in range(4)]
            v2t = [sub.tile([PC, NF], BF16, tag="v2t", bufs=KC,
                            name=f"v2t{i}") for i in range(KC)]
            x3T = sub.tile([PC, KC * NF], BF16, tag="x3T")
            stsb2 = sub.tile([33, NF], FP32, tag="lnst", name="stsb2")

            def after_band2(t4, otile):
                bnd = x3T[:].rearrange("p (j r) -> p j r", j=KC)[
                    :, :, PC * t4:PC * t4 + PC]
                nob = nTo[:].rearrange("p (j r) -> p j r", j=KC)[
                    :, :, PC * t4:PC * t4 + PC]
                nc.vector.tensor_tensor(bnd, bnd, nob, op=ALU.add)
                _band_stats(nc, tc, sub, x3T[:], t4, otile, stsb2, c, "b2")

            wv2ts = []
            for kc in range(KC):
                wt = sub.tile([PC, NF], BF16, tag="wv_v2", bufs=KC,
                              name=f"wv_v2{kc}")
                nc.sync.dma_start(wt[:], dram["wv2"].ap()[kc])
                wv2ts.append(wt)

            with tc.tile_pool(name="s2p", space="PSUM", bufs=1) as psp:
                # own-row halves of k2/v2 run from nTo while the
                # AllGather is in flight
                _proj_v(nc, sub, psp, wv2ts, bv2B,
                        lambda kc, pc: nTo[:, kc * NF + pc * PC:
                                           kc * NF + (pc + 1) * PC],
                        v2t, "v2o", pc_range=(0, 4))
                k2w = [sub.tile([PC, KC, PC], BF16, tag="w_k2", bufs=4,
                                name=f"wk2_{m}") for m in range(4)]
                for m in range(4):
                    nc.sync.dma_start(k2w[m][:], dram["wk2"].ap()[m])
                for m in range(4):
                    ps = psp.tile([PC, NF], FP32, tag="proj", bufs=2,
                                  name=f"k2o_{m}")
                    for kc in range(KC):
                        nc.tensor.matmul(
                            ps[:], k2w[m][:, kc, :],
                            nTo[:, kc * NF:kc * NF + NF],
                            start=(kc == 0), stop=(kc == KC - 1))
                    nc.scalar.activation(k2T[m][:, 0:NF], ps[:],
                                         AF.Identity,
                                         bias=c["bk2c"][:, m:m + 1])

                # keep PE warm through the gather window
                wp2 = psp.tile([1, NF], FP32, tag="proj", bufs=2,
                               name="cwarm")
                for i in range(64):
                    nc.tensor.matmul(wp2[:], c["onesd"][:], k2T[0][:, 0:NF],
                                     start=(i == 0), stop=(i == 63))

                # gathered blocks (global row order) + exact partner
                # recovery: partner = (block0 - own) + block1
                nTg = [sub.tile([PC, N], BF16, tag="nTg", bufs=KC,
                                name=f"nTg{i}") for i in range(KC)]
                for j in range(KC):
                    for r in range(2):
                        nc.sync.dma_start(
                            nTg[j][:, r * NF:(r + 1) * NF],
                            ag_out.ap()[r, j * PC:(j + 1) * PC])
                nTp = [sub.tile([PC, NF], BF16, tag="nTp", bufs=KC,
                                name=f"nTp{i}") for i in range(KC)]
                for j in range(KC):
                    tdif = sub.tile([PC, NF], FP32, tag="tdif", bufs=4,
                                    name=f"tdif{j}")
                    nc.vector.tensor_tensor(
                        tdif[:], nTg[j][:, 0:NF],
                        nTo[:, j * NF:(j + 1) * NF], op=ALU.subtract)
                    nc.vector.tensor_tensor(
                        nTp[j][:], tdif[:], nTg[j][:, NF:N], op=ALU.add)

                # rest of the MLP fc1 weights stream in the background
                for f in range(8, FT):
                    pass  # prefetched in stage 3 (SBUF budget)

                q2_of = lambda kc, nf: nTg[kc][:, nf * NF:(nf + 1) * NF]
                all_pts = {}
                for m in range(4):
                    # partner half of k2 for this m
                    ps = psp.tile([PC, NF], FP32, tag="proj", bufs=2,
                                  name=f"k2p_{m}")
                    for kc in range(KC):
                        nc.tensor.matmul(
                            ps[:], k2w[m][:, kc, :], nTp[kc][:],
                            start=(kc == 0), stop=(kc == KC - 1))
                    nc.scalar.activation(k2T[m][:, NF:N], ps[:],
                                         AF.Identity,
                                         bias=c["bk2c"][:, m:m + 1])
                    _proj_T(nc, sub, psp, dram["wq2"], c["bq2c"], q2_of,
                            q2T, "q2", m_range=(m, m + 1))
                    if m < 2:
                        for qh in range(2):
                            all_pts[(m, qh)] = _attn_scores(
                                nc, sub, psp, q2T, k2T, m, qh, "y")
                    elif m == 2:
                        _proj_v(nc, sub, psp, wv2ts, bv2B,
                                lambda kc, pc: nTp[kc][
                                    :, (pc - 4) * PC:(pc - 3) * PC],
                                v2t, "v2p", pc_range=(4, 8))
                for t4 in range(2, 4):
                    for qh in range(2):
                        all_pts[(t4, qh)] = _attn_scores(
                            nc, sub, psp, q2T, k2T, t4, qh, "y")
                for t4 in range(4):
                    _attn_group(nc, sub, psp, v2t,
                                [all_pts[(t4, 0)], all_pts[(t4, 1)]],
                                x3T[:], t4, c, "y",
                                after_band=after_band2)

            mub2, rstdb2 = _ln_tail(nc, sub, stsb2, c, "ln2")

            def ln2_out(j, t2):
                nc.scalar.activation(
                    n3T[:, j * NF:(j + 1) * NF], t2[:], AF.Identity,
                    bias=c["betac"][:, j:j + 1],
                    scale=c["gammac"][:, j:j + 1])

            _ln_apply(nc, sub, x3T[:], mub2, rstdb2, c, "ln2", ln2_out)
            if dbg:
                nc.sync.dma_start(dbg["dx3T"].ap(), x3T[:])
                nc.sync.dma_start(dbg["dn3T"].ap(), n3T[:])

        # ---- stage 3: MLP ----
        with tc.tile_pool(name="s3", bufs=1) as sub:
            hT = [sub.tile([PC, NF], BF16, tag="hT", bufs=FT,
                           name=f"hT{i}") for i in range(FT)]
            w2pre = [sub.tile([PC, FT, PC], BF16, tag="w2t", bufs=8,
                              name=f"w2t{d}") for d in range(KC)]
            with tc.tile_pool(name="s3p", space="PSUM", bufs=1) as psp:
                for f in range(FT):
                    wt = sub.tile([PC, KC, PC], BF16, tag="w1t", bufs=10,
                                  name=f"w1t{f}")
                    nc.sync.dma_start(wt[:], dram["w1"].ap()[f])
                    if f >= 8 and (f - 8) % 3 == 0 and (f - 8) // 3 < KC:
                        nc.sync.dma_start(w2pre[(f - 8) // 3][:],
                                          dram["w2"].ap()[(f - 8) // 3])
                    ps = psp.tile([PC, NF], FP32, tag="mlp", bufs=8,
                                  name=f"h{f}")
                    for kc in range(KC):
                        nc.tensor.matmul(
                            ps[:], wt[:, kc, :],
                            n3T[:, kc * NF:(kc + 1) * NF],
                            start=(kc == 0), stop=(kc == KC - 1))
                    nc.scalar.activation(hT[f][:], ps[:], AF.Gelu,
                                         bias=c["b1c"][:, f:f + 1])
                for d in range(KC):
                    ps = psp.tile([PC, NF], FP32, tag="mlp", bufs=8,
                                  name=f"yp{d}")
                    for f in range(FT):
                        nc.tensor.matmul(ps[:], w2pre[d][:, f, :], hT[f][:],
                                         start=(f == 0), stop=(f == FT - 1))
                    yt = sub.tile([PC, NF], FP32, tag="yT", bufs=4,
                                  name=f"yT{d}")
                    nc.vector.scalar_tensor_tensor(
                        yt[:], ps[:], c["b2c"][:, d:d + 1],
                        n3T[:, d * NF:(d + 1) * NF],
                        op0=ALU.add, op1=ALU.add)
                    nc.sync.dma_start(
                        y_out.ap()[d * PC:(d + 1) * PC], yt[:])


def _get_nc():
    if "nc" not in _CACHE:
        _CACHE["nc"] = _build()
    return _CACHE["nc"]


def _prep_inputs(inputs):
    """Host-side slicing/transposition into per-core bf16 DRAM layouts."""
    f32 = np.float32
    x1 = np.ascontiguousarray(np.asarray(inputs["x1"], f32))
    x2 = np.ascontiguousarray(np.asarray(inputs["x2"], f32))
    Wq = np.asarray(inputs["Wq"], f32)
    Wkv = np.asarray(inputs["Wkv"], f32)
    Wqkv = np.asarray(inputs["Wqkv"], f32)
    W1 = np.asarray(inputs["W1"], f32)
    W2 = np.asarray(inputs["W2"], f32)
    bq = np.asarray(inputs["bq"], f32)
    bkv = np.asarray(inputs["bkv"], f32)
    bqkv = np.asarray(inputs["bqkv"], f32)
    gamma = np.asarray(inputs["gamma"], f32)
    beta = np.asarray(inputs["beta"], f32)
    b1 = np.asarray(inputs["b1"], f32)
    b2 = np.asarray(inputs["b2"], f32)

    def wcols(Wslice):     # (1024, 512) -> (4, 128, 8, 128) bf16
        return np.ascontiguousarray(
            Wslice.reshape(KC, PC, 4, PC).transpose(2, 1, 0, 3)).astype(BF)

    def bcols(bslice, n):  # (n*128,) -> (128, n) fp32
        return np.ascontiguousarray(bslice.reshape(n, PC).T)

    w1h = np.ascontiguousarray(
        W1.reshape(KC, PC, FT, PC).transpose(2, 1, 0, 3)).astype(BF)
    w2h = np.ascontiguousarray(
        W2.reshape(FT, PC, KC, PC).transpose(2, 1, 0, 3)).astype(BF)
    b1h = bcols(b1, FT)
    b2h = bcols(b2, KC)
    gh = bcols(gamma, KC)
    bh = bcols(beta, KC)

    in_maps = []
    for core in range(8):
        b, hh = core // 2, core % 2
        lo = NF * hh
        x2t = np.ascontiguousarray(x2[b].T)
        x1t = np.ascontiguousarray(x1[b].T)
        x2own = np.ascontiguousarray(
            x2t[:, lo:lo + NF].reshape(KC, PC, NF).transpose(1, 0, 2)
            .reshape(PC, KC * NF)).astype(BF)
        in_maps.append({
            "x2t": x2t.astype(BF), "x1t": x1t.astype(BF), "x2own": x2own,
            "wq": wcols(Wq[:, lo:lo + NF]),
            "wk": wcols(Wkv[:, lo:lo + NF]),
            "wv": np.ascontiguousarray(
                Wkv[:, D + lo:D + lo + NF].reshape(KC, PC, NF)).astype(BF),
            "wq2": wcols(Wqkv[:, lo:lo + NF]),
            "wk2": wcols(Wqkv[:, D + lo:D + lo + NF]),
            "wv2": np.ascontiguousarray(
                Wqkv[:, 2 * D + lo:2 * D + lo + NF]
                .reshape(KC, PC, NF)).astype(BF),
            "w1": w1h, "w2": w2h,
            "bqc": bcols(bq[lo:lo + NF], 4),
            "bkc": bcols(bkv[lo:lo + NF], 4),
            "bq2c": bcols(bqkv[lo:lo + NF], 4),
            "bk2c": bcols(bqkv[D + lo:D + lo + NF], 4),
            "bvr": np.ascontiguousarray(
                bkv[D + lo:D + lo + NF].reshape(1, NF)),
            "bv2r": np.ascontiguousarray(
                bqkv[2 * D + lo:2 * D + lo + NF].reshape(1, NF)),
            "b1c": b1h, "b2c": b2h, "gammac": gh, "betac": bh,
        })
    return in_maps


def kernel(**inputs):
    in_maps = _prep_inputs(inputs)
    nc = _get_nc()
    res = run_bass_kernel_spmd(nc, in_maps, core_ids=list(range(8)))
    _CACHE["last_results"] = res
    out = np.zeros((B, N, D), np.float32)
    for core in range(8):
        b, hh = core // 2, core % 2
        out[b, NF * hh:NF * hh + NF, :] = res.results[core]["y"].T
    return out
